# revision 64
# baseline (speedup 1.0000x reference)
"""Trainium2 Bass kernel for nn_DimeNetBlock (gnn_message_passing).

Algorithm notes (derived from the reference):
- compute_angle(v, -v) makes the coord path degenerate: for every non-self-
  loop edge the angle is arccos(clip(-(vn.vn))) ~= pi; pi*W1[16] is folded
  into the first-layer bias and only the tiny per-edge deviation (or -pi/2
  for self-loops) travels with the edge features, so no coord gather is
  needed on device.
- The second edge-MLP linear (W2) commutes with the segment sum and is
  folded into the first update-MLP linear: agg@W2@W3 = (sum SiLU(h1)) @ W23.
- Edges are sorted by destination node; nodes are sharded across the 8
  cores (12500 each) so no collective is needed. Within a core, nodes go to
  4 streams x degree-class grids (classes every 2, small classes merged so
  every region has >= MIN_M nodes, capacities even for AP alignment) with
  pad slots, so the device pipeline is dense and index-free:
    matmul1 (block-diag W1, fp8, K=128) -> SiLU -> pairwise-halving segment
    tree (fp16) -> update MLP (block-diag) -> + x -> output.
- The first-layer bias rides inside the fp8 matmul as two extra lhsT rows
  (r1 = fp8(b1'), r2 = fp8(residual)); ScalarE's activation bias carries
  only the ~1e-3 quantization residual, so the ACT path stays exact.
- SiLU is split between ScalarE (the pacing engine; exact table eval) and
  VectorE: every DVE_EVERY-th PSUM chunk runs a custom single-instruction
  DVE op f(x) = x*clamp01(c0 + c1*x + c2*x^3) (8 ALU stages, per-embed-dim
  coefficients via s0/s1/Latch(Src1), registered into concourse.dve_ops at
  import). Coefficients are fitted at prepare() time against the Gaussian
  h-distribution h_j ~ N(b1'_j, ||W1[:,j]||) with a 6-sigma tail anchor.
  Pad columns of DVE chunks zero the pad-flag AND the bias rows so x=0
  exactly and f(0)=0 structurally.
- The E and O segment trees of one group share a single [128, 2*Fg] fp16
  tile and run as rank-3 [128, 2, w] tensor_adds (row stride Fg), halving
  tree-instruction count; all region offsets stay even so the 2x f16 DVE
  mode engages everywhere. Trees are emitted with one-group hysteresis so
  they never head-block the in-order engine queues.
- Phase A uses 1024-col PSUM chunks with 3 pool buffers (6 banks) so the
  matmul->activation ping-pong stays decoupled; phase B (update MLP, f16
  weights) runs on its own 512-col PSUM ring interleaved with two-group
  hysteresis; the final +x (xw = x + b4) is a fused vector add into a
  [128, 2*M_tot] output tile, DMA'd out at 2048-col milestones.
- Host gathers/transposes the per-core feature-major outputs at the end.
"""

import numpy as np
import ml_dtypes

try:
    import concourse.bass as bass  # noqa: F401
except Exception:  # pragma: no cover
    import sys

    sys.path.insert(0, "/opt/trn_rl_repo")
import concourse.tile as tile
from concourse import bacc, mybir
from concourse.bass_utils import run_bass_kernel_spmd

BF16 = ml_dtypes.bfloat16
FP8 = ml_dtypes.float8_e4m3fn

N_NODES = 100000
E_EDGES = 3200000
EMBED = 64
N_CORES = 8
NPC = N_NODES // N_CORES
N_STREAMS = 4
PAD_W = -240.0
PAD_FLAG_DVE = 0.0  # DVE-chunk pads produce x=0 exactly; f(0)=0 structurally
SUBREGION_COLS = 7680
PSUM_CHUNK = 1024
MM_N = 512
DVE_EVERY = 5  # every k-th (group, pass) psum chunk runs SiLU on VectorE
MIN_M = 48  # merge degree classes until every region has >= this many nodes
POOL_TREE_FRAC = 0.0  # fraction of tree columns assigned to GpSimd

TRACE = False  # set by test.py to capture a profile


# --------------------------------------------------------------------------
# custom DVE op: f(x) = x * clamp(c0 + c1*x + c2*x^3, 0, 1)
# --------------------------------------------------------------------------

def _register_dve_silu():
    """f(x) = x * clamp(c0 + c1*x + c2*x^3, 0, imm2), imm2 = 1.0.
    c0 = s0 (AP [P,1]), c1 = s1 (AP), c2 = in1 [P,1] via Latch. The
    first-layer bias arrives in x itself (two fp8 lhsT rows), so x is the
    full pre-activation."""
    import concourse.dve_ops as dve_ops
    from concourse.dve_ops import DveOp
    from concourse.dve_spec import (
        Spec, Src0, Src1, C0, C1, C2, Zero, Latch, lower, maxx, minn,
    )
    from concourse.dve_uop import DveOpSpec

    name = "SILU_CLAMP_CUBIC_ANT"
    if name in dve_ops._SUB_OPCODE_FOR_NAME:
        return next(op for op in dve_ops.OPS if op.name == name)

    def _ref(in0, in1, s0, s1, imm2):
        x = in0.astype(np.float32)
        P = x.shape[0]
        c2 = np.asarray(in1, np.float32).reshape(P, 1)
        c0 = np.asarray(s0, np.float32).reshape(P, 1) if hasattr(s0, "reshape") else s0
        c1 = np.asarray(s1, np.float32).reshape(P, 1) if hasattr(s1, "reshape") else s1
        x2 = x.reshape(P, -1)
        q = c0 + c1 * x2 + c2 * x2 * x2 * x2
        return (x2 * np.clip(q, 0.0, imm2)).reshape(x.shape).astype(np.float32)

    c2 = Latch(Src1)
    u = c2 * (Src0 * Src0)
    u = u + C1
    u = u * Src0
    u = u + C0
    u = maxx(u, Zero)
    u = minn(u, C2)  # C2 = imm2 literal (1.0)
    spec = Spec(body=u * Src0, reference=_ref)
    row = dve_ops._CUSTOM_DVE_ROW_BASE + len(dve_ops.OPS)
    assert row < 0x20
    shas = {}
    for ver in ("v3", "v4"):
        try:
            uops = lower(spec, ver=ver)
            s = DveOpSpec(name=name, opcode=row, uops=uops, rd1_en=True)
            shas[ver] = s.sha(ver)
        except Exception:
            pass
    op = DveOp(name, spec, subdim=False, uops_sha=shas)
    dve_ops.OPS.append(op)
    dve_ops.CUSTOM_DVE_SPECS[name] = spec
    dve_ops._SUB_OPCODE_FOR_NAME[name] = row
    return op


SILU_DVE_OP = _register_dve_silu()


def _fit_silu_coefs(W1, b1):
    """Per-embed-dim (c0, c1, c2) for x*clamp01(c0+c1*x+c2*x^3) ~ silu(x)
    under x ~ N(mu_j, sigma_j). Pads are exact zeros by construction, so
    the fit is unconstrained apart from a light tail-error cap."""
    mu = (b1.astype(np.float64) + np.pi * W1[16].astype(np.float64))
    sg = np.sqrt((W1[:16].astype(np.float64) ** 2).sum(0))
    FLOOR = 1e-4  # tail-anchor weight (keeps the cubic near sigma to 6 sigma)
    out = np.zeros((EMBED, 3), np.float64)
    for j in range(EMBED):
        x = np.linspace(mu[j] - 5.2 * sg[j], mu[j] + 5.2 * sg[j], 301)
        pdf = np.exp(-0.5 * ((x - mu[j]) / sg[j]) ** 2)
        pdf /= pdf.sum()
        xt = np.concatenate([
            np.linspace(mu[j] - 6.0 * sg[j], mu[j] - 3.0 * sg[j], 30),
            np.linspace(mu[j] + 3.0 * sg[j], mu[j] + 6.0 * sg[j], 30),
        ])
        w = np.sqrt(pdf) * (np.abs(x) + 0.1)
        wt = np.sqrt(FLOOR) * (np.abs(xt) + 0.1)
        sig = 1 / (1 + np.exp(-x))
        sigt = 1 / (1 + np.exp(-xt))
        m = np.ones_like(x, bool)
        sol = np.array([0.5, 0.25, -0.005])
        for _ in range(5):
            xx = np.concatenate([x[m], xt])
            ww = np.concatenate([w[m], wt])
            ss = np.concatenate([sig[m], sigt])
            A = np.stack([np.ones_like(xx), xx, xx ** 3], 1)
            sol, *_ = np.linalg.lstsq(A * ww[:, None], ss * ww, rcond=None)
            q = sol[0] + sol[1] * x + sol[2] * x ** 3
            newm = (q > 0.02) & (q < 0.98)
            if newm.sum() < 8:
                newm = np.ones_like(x, bool)
            if (newm == m).all():
                break
            m = newm
        out[j] = sol
    return out.astype(np.float32)


# --------------------------------------------------------------------------
# host-side layout
# --------------------------------------------------------------------------

def _compute_angle(coord, row, col):
    v = (coord[row] - coord[col]).astype(np.float32)
    norm = np.maximum(
        np.sqrt((v * v).sum(-1, keepdims=True)).astype(np.float32), np.float32(1e-12)
    )
    vn = (v / norm).astype(np.float32)
    cos = (-(vn * vn).sum(-1)).astype(np.float32)
    lo = np.float32(-1.0 + 1e-8)
    hi = np.float32(1.0 - 1e-8)
    return np.arccos(np.clip(cos, lo, hi)).astype(np.float32)


def _make_layout(deg):
    """Degree classes every 2; per-core nodes are dealt degree-descending
    round-robin into 4 streams, class capacities are the minimal
    suffix-feasible values over all (core, stream) pairs (rounded to even so
    all region offsets stay even), and each stream fills its class slots in
    degree order."""
    dmax = int(deg.max())
    classes_D = list(range(8, dmax + 2, 2))
    while classes_D[-1] < dmax:
        classes_D.append(classes_D[-1] + 2)
    n_cls = len(classes_D)
    Darr = np.array(classes_D)

    cls_of_deg = np.searchsorted(Darr, np.arange(dmax + 1))
    node_cls = cls_of_deg[deg]

    per_cq_nodes = []  # [core][q] -> node array sorted by deg desc
    suffix_req = np.zeros(n_cls + 1, np.int64)
    for c in range(N_CORES):
        lo, hi = c * NPC, (c + 1) * NPC
        order = np.argsort(-deg[lo:hi], kind="stable") + lo
        qs = [order[q::N_STREAMS] for q in range(N_STREAMS)]
        per_cq_nodes.append(qs)
        for q in range(N_STREAMS):
            kcnt = np.bincount(node_cls[qs[q]], minlength=n_cls)
            sfx = np.cumsum(kcnt[::-1])[::-1]
            suffix_req[:n_cls] = np.maximum(suffix_req[:n_cls], sfx)

    # minimal capacities: M_k = max(0, R_k - sum_{j>k} M_j); forced even
    M = [0] * n_cls
    tail = 0
    for k in range(n_cls - 1, -1, -1):
        M[k] = max(0, int(suffix_req[k]) - tail)
        if M[k] % 2:
            M[k] += 1
        tail += M[k]

    # merge small classes upward (into the next larger D) so no region has
    # fewer than MIN_M nodes: kills the long tail of tiny tree ops at a
    # small padding cost
    for k in range(n_cls - 1):
        if 0 < M[k] < MIN_M:
            M[k + 1] += M[k]
            M[k] = 0
    # if the largest class ended up small, fold it into the nearest
    # nonzero class below (its nodes just get a bigger D)
    k = n_cls - 1
    if 0 < M[k] < MIN_M:
        j = k - 1
        while j >= 0 and M[j] == 0:
            j -= 1
        if j >= 0:
            # move capacity up from class j instead: nodes of class j can't
            # go down, so merge j upward into k
            M[k] += M[j]
            M[j] = 0

    # per (core, stream): fill class slots from the largest class down with
    # the highest-degree remaining nodes
    per_core_cls_nodes = []
    for c in range(N_CORES):
        entry = [[None] * N_STREAMS for _ in range(n_cls)]
        for q in range(N_STREAMS):
            nodes = per_cq_nodes[c][q]  # deg desc
            idx = 0
            for k in range(n_cls - 1, -1, -1):
                take = nodes[idx : idx + M[k]]
                entry[k][q] = take
                idx += len(take)
            assert idx == len(nodes)
        per_core_cls_nodes.append([tuple(entry[k]) for k in range(n_cls)])

    regions = []
    col_off = 0
    node_off = 0
    for k in range(n_cls - 1, -1, -1):
        D = classes_D[k]
        if M[k] == 0:
            continue
        m_max = max(2, (SUBREGION_COLS // D) & ~1)
        n_parts = -(-M[k] // m_max)
        m_even = -(-M[k] // n_parts)
        m_even += m_even % 2
        left = M[k]
        while left > 0:
            m = min(m_even, left)
            regions.append((D, m, col_off, node_off))
            col_off += D * m
            node_off += m
            left -= m

    # group contiguous regions into shared load tiles; caps grow so the
    # pipeline ramps quickly
    groups = []
    cur, curF = [], 0
    cap_seq = [768, 1536, 3072, 4608]
    total_cols = sum(D * m for D, m, _, _ in regions)
    seen = 0
    for reg in regions:
        F = reg[0] * reg[1]
        if len(groups) < len(cap_seq):
            cap = cap_seq[len(groups)]
        else:
            cap = SUBREGION_COLS
        if curF + F > cap and cur:
            groups.append((cur, curF))
            cur, curF = [], 0
        cur.append(reg)
        curF += F
        seen += F
    if cur:
        groups.append((cur, curF))

    # assign whole-region trees to GpSimd at a fixed ratio, spread evenly
    # across the emission sequence so Pool's load tracks the pipeline
    pool_regions = set()
    pool_cols = 0
    tot = 0
    for i, (D, m, _, _) in enumerate(regions):
        tot += D * m
        if pool_cols + D * m <= POOL_TREE_FRAC * tot:
            pool_regions.add(i)
            pool_cols += D * m

    return {
        "classes_D": classes_D,
        "class_M": M,
        "regions": regions,
        "groups": groups,
        "pool_regions": pool_regions,
        "S": col_off,
        "M_tot": node_off,
        "per_core_cls_nodes": per_core_cls_nodes,
    }


def _chunk_plan(layout):
    """(gi, pass E=0/O=1, c0, cw, on_dve) for every psum chunk, in emission
    order. Used by both the host array builder (pad-flag values) and the
    device builder (engine selection)."""
    plan = []
    n = 0
    for gi, (grp, Fg) in enumerate(layout["groups"]):
        for p in (0, 1):
            for c0 in range(0, Fg, PSUM_CHUNK):
                cw = min(PSUM_CHUNK, Fg - c0)
                on_dve = (n % DVE_EVERY) == (DVE_EVERY - 1)
                plan.append((gi, p, c0, cw, on_dve))
                n += 1
    return plan


def _build_host_arrays(x, coord, rbf_feature, edge_index, W1, b1, b4, layout):
    row = np.ascontiguousarray(edge_index[0]).astype(np.int64)
    col = np.ascontiguousarray(edge_index[1]).astype(np.int64)
    angle = _compute_angle(coord, row, col)

    regions = layout["regions"]
    pccn = layout["per_core_cls_nodes"]
    S, M_tot = layout["S"], layout["M_tot"]
    classes_D = layout["classes_D"]
    n_cls = len(classes_D)

    cls_regions = [[] for _ in range(n_cls)]
    for D, m, co, no in regions:
        cls_regions[classes_D.index(D)].append((m, co, no))

    node_stream = np.full(N_NODES, -1, np.int8)
    node_colbase = np.zeros(N_NODES, np.int64)
    node_m = np.zeros(N_NODES, np.int64)
    node_li = np.zeros(N_NODES, np.int64)
    node_pos = np.zeros(N_NODES, np.int64)
    for c in range(N_CORES):
        for k in range(n_cls):
            for q in range(N_STREAMS):
                nodes = pccn[c][k][q]
                if len(nodes) == 0:
                    continue
                idx = 0
                for m, co, no in cls_regions[k]:
                    take = nodes[idx : idx + m]
                    if len(take) == 0:
                        break
                    li = np.arange(len(take))
                    node_stream[take] = q
                    node_colbase[take] = co
                    node_m[take] = m
                    node_li[take] = li
                    node_pos[take] = no + li
                    idx += m

    order = np.argsort(row, kind="stable")
    row_s = row[order]
    deg = np.bincount(row_s, minlength=N_NODES)
    starts = np.zeros(N_NODES + 1, np.int64)
    np.cumsum(deg, out=starts[1:])
    pos_in_seg = np.arange(E_EDGES) - starts[row_s]
    ecol = node_colbase[row_s] + pos_in_seg * node_m[row_s] + node_li[row_s]
    equarter = node_stream[row_s]
    ecore = row_s // NPC

    rbfT_s = np.ascontiguousarray(rbf_feature.T).astype(FP8)[:, order]
    # angle deviation from pi (pi*W1[16] is folded into bias1 exactly)
    angle_s = (angle[order].astype(np.float64) - np.pi).astype(np.float32).astype(FP8)

    # per-(column, pass) pad-flag value: DVE chunks use the scaled flag
    groups = layout["groups"]
    group_co0 = []
    off = 0
    for grp, Fg in groups:
        group_co0.append(off)
        off += Fg
    flag = np.full((2, S), 1.0, np.float32)  # [pass, col]
    for gi, p, c0, cw, on_dve in _chunk_plan(layout):
        if on_dve:
            a = group_co0[gi] + c0
            flag[p, a : a + cw] = PAD_FLAG_DVE
    flag8 = flag.astype(FP8)

    rbf_devs, xw_devs = [], []
    for c in range(N_CORES):
        dev = np.zeros((128, S), dtype=FP8)
        for q in range(N_STREAMS):
            dev[32 * q + 17, :] = flag8[q // 2]  # pad flag (per-pass value)
            sel = (ecore == c) & (equarter == q)
            cols = ecol[sel]
            r0 = 32 * q
            dev[r0 : r0 + 16, cols] = rbfT_s[:, sel]
            dev[r0 + 16, cols] = angle_s[sel]
            dev[r0 + 17, cols] = FP8(0.0)
            # bias rows: on for real edges only, so DVE-chunk pads
            # (flag 0 there) yield x = 0 exactly
            dev[r0 + 18, cols] = FP8(1.0)
            dev[r0 + 19, cols] = FP8(1.0)
        rbf_devs.append(dev)

        # xw: [128, 2*M_tot] -- cols [0,M_tot) streams A|B, [M_tot,2M) C|D
        xw = np.zeros((128, 2 * M_tot), np.float32)
        nodes = np.arange(c * NPC, (c + 1) * NPC)
        xb = (x[nodes].astype(np.float32) + b4[None, :].astype(np.float32)).T
        st = node_stream[nodes]
        ps = node_pos[nodes]
        for q in range(N_STREAMS):
            rows = slice(0, 64) if q % 2 == 0 else slice(64, 128)
            off = 0 if q < 2 else M_tot
            xw[rows, off + ps[st == q]] = xb[:, st == q]
        xw_devs.append(xw.astype(BF16))

    # first-layer bias delivered through the fp8 matmul as two rows
    # (r1 = fp8(b1'), r2 = fp8(residual)): quantization residual ~7e-4
    b1a = (b1.astype(np.float64) + np.pi * W1[16].astype(np.float64)).astype(
        np.float32
    )
    r1 = b1a.astype(FP8)
    r2 = (b1a - r1.astype(np.float32)).astype(FP8)
    bhat = (r1.astype(np.float32) + r2.astype(np.float32)).astype(np.float32)
    blk = np.concatenate(
        [
            W1[:16].astype(np.float32),
            W1[16:17].astype(np.float32),
            np.full((1, EMBED), PAD_W, np.float32),
            r1.astype(np.float32)[None, :],
            r2.astype(np.float32)[None, :],
        ],
        axis=0,
    )  # [20, 64]
    lhsT_E = np.zeros((128, 128), np.float32)
    lhsT_E[0:20, 0:64] = blk  # stream A (rows 0-19) -> out cols 0-63
    lhsT_E[32:52, 64:128] = blk  # stream B -> out cols 64-127
    lhsT_O = np.zeros((128, 128), np.float32)
    lhsT_O[64:84, 0:64] = blk  # stream C
    lhsT_O[96:116, 64:128] = blk  # stream D

    meta = {"node_stream": node_stream, "node_pos": node_pos, "deg": deg}
    return rbf_devs, xw_devs, lhsT_E.astype(FP8), lhsT_O.astype(FP8), meta, bhat


def _blockdiag2(W):
    out = np.zeros((128, 128), np.float32)
    out[0:64, 0:64] = W
    out[64:128, 64:128] = W
    return out


# --------------------------------------------------------------------------
# device kernel
# --------------------------------------------------------------------------

def _emit_tree(nc, s3, co, sum3, D, m, no, f16_add):
    """Pairwise-halving segment reduction along the slot axis, fused over
    the E/O halves via rank-3 [128, 2, w] APs (row stride Fg)."""
    dd = D
    while dd > 1:
        if dd % 2 == 1:
            f16_add(
                s3[:, :, co : co + m],
                s3[:, :, co : co + m],
                s3[:, :, co + (dd - 1) * m : co + dd * m],
            )
            dd -= 1
        elif dd == 2:
            f16_add(
                sum3[:, :, no : no + m],
                s3[:, :, co : co + m],
                s3[:, :, co + m : co + 2 * m],
            )
            dd = 1
        else:
            half = (dd // 2) * m
            f16_add(
                s3[:, :, co : co + half],
                s3[:, :, co : co + half],
                s3[:, :, co + half : co + 2 * half],
            )
            dd //= 2


def _build_nc(layout, b2W3_nonzero):
    S, M_tot = layout["S"], layout["M_tot"]
    f16 = mybir.dt.float16
    bf16 = mybir.dt.bfloat16
    f32 = mybir.dt.float32
    f8 = mybir.dt.float8e4
    SILU = mybir.ActivationFunctionType.Silu

    plan = _chunk_plan(layout)
    plan_by_g = {}
    for gi, p, c0, cw, on_dve in plan:
        plan_by_g.setdefault((gi, p), []).append((c0, cw, on_dve))

    nc = bacc.Bacc("TRN2", target_bir_lowering=False, debug=False, num_devices=N_CORES)
    rbf_d = nc.dram_tensor("rbf_dev", [128, S], f8, kind="ExternalInput")
    xw_d = nc.dram_tensor("xw_dev", [128, 2 * M_tot], bf16, kind="ExternalInput")
    lhsE_d = nc.dram_tensor("lhsT_E", [128, 128], f8, kind="ExternalInput")
    lhsO_d = nc.dram_tensor("lhsT_O", [128, 128], f8, kind="ExternalInput")
    bias1_d = nc.dram_tensor("bias1", [128, 1], f32, kind="ExternalInput")
    scoef_d = nc.dram_tensor("scoef", [128, 3], f32, kind="ExternalInput")
    wup_d = nc.dram_tensor("Wup", [128, 128], f16, kind="ExternalInput")
    bias3_d = nc.dram_tensor("bias3", [128, 1], f32, kind="ExternalInput")
    w4_d = nc.dram_tensor("W4x2", [128, 128], f16, kind="ExternalInput")
    if b2W3_nonzero:
        degb_d = nc.dram_tensor("deg_dev", [4, 2 * M_tot], f16, kind="ExternalInput")
        b2w3_d = nc.dram_tensor("b2w3cat", [2, 128], f16, kind="ExternalInput")
    out_d = nc.dram_tensor("out_dev", [128, 2 * M_tot], bf16, kind="ExternalOutput")

    with tile.TileContext(nc) as tc:
        with (
            tc.tile_pool(name="const", bufs=1) as cpool,
            tc.tile_pool(name="rbf", bufs=5) as rbf_pool,
            tc.tile_pool(name="s", bufs=4) as s_pool,
            tc.tile_pool(name="psum", bufs=3, space="PSUM") as psum_pool,
            tc.tile_pool(name="psumB", bufs=2, space="PSUM") as psumB_pool,
            tc.tile_pool(name="b", bufs=4) as b_pool,
        ):
            # tiny dummy activation up front: forces the Silu table load
            # to overlap the input DMAs instead of gating the first real ACT
            warm = cpool.tile([1, 8], f32)
            nc.scalar.memzero(warm[:])
            nc.scalar.activation(
                warm[:], warm[:], mybir.ActivationFunctionType.Silu
            )
            lhsE = cpool.tile([128, 128], f8)
            nc.sync.dma_start(out=lhsE[:], in_=lhsE_d[:, :])
            lhsO = cpool.tile([128, 128], f8)
            nc.sync.dma_start(out=lhsO[:], in_=lhsO_d[:, :])
            bias1 = cpool.tile([128, 1], f32)
            nc.sync.dma_start(out=bias1[:], in_=bias1_d[:, :])
            scoef = cpool.tile([128, 3], f32)
            nc.sync.dma_start(out=scoef[:], in_=scoef_d[:, :])
            sum2 = cpool.tile([128, 2 * M_tot], f16)
            out_sb = cpool.tile([128, 2 * M_tot], bf16)
            sum3 = sum2[:].rearrange("p (h f) -> p h f", h=2)
            sum_AB = sum2[:, 0:M_tot]
            sum_CD = sum2[:, M_tot : 2 * M_tot]

            def f16_add(out, a, b):
                nc.vector.tensor_add(out=out, in0=a, in1=b)

            def f16_add_pool(out, a, b):
                nc.gpsimd.tensor_add(out=out, in0=a, in1=b)

            def emit_b_chunk(half, t0, w):
                sum_t = sum2
                base = half * M_tot
                ps3 = psumB_pool.tile([128, w], f32, tag="psB")
                for m0 in range(0, w, MM_N):
                    mw = min(MM_N, w - m0)
                    nc.tensor.matmul(
                        out=ps3[:, m0 : m0 + mw],
                        lhsT=wup[:],
                        rhs=sum_t[:, base + t0 + m0 : base + t0 + m0 + mw],
                        start=True,
                        stop=not b2W3_nonzero,
                    )
                    if b2W3_nonzero:
                        nc.tensor.matmul(
                            out=ps3[:, m0 : m0 + mw],
                            lhsT=b2w3[0:1, :],
                            rhs=degb[
                                2 * half : 2 * half + 1,
                                base + t0 + m0 : base + t0 + m0 + mw,
                            ],
                            start=False,
                            stop=False,
                        )
                        nc.tensor.matmul(
                            out=ps3[:, m0 : m0 + mw],
                            lhsT=b2w3[1:2, :],
                            rhs=degb[
                                2 * half + 1 : 2 * half + 2,
                                base + t0 + m0 : base + t0 + m0 + mw,
                            ],
                            start=False,
                            stop=True,
                        )
                s3 = b_pool.tile([128, w], f16, tag="s3")
                nc.scalar.activation(s3[:, :w], ps3[:, :w], SILU, bias=bias3[:])
                return s3

            def emit_b_tail(half, t0, w, s3):
                base = half * M_tot
                ps4 = psumB_pool.tile([128, w], f32, tag="psB")
                for m0 in range(0, w, MM_N):
                    mw = min(MM_N, w - m0)
                    nc.tensor.matmul(
                        out=ps4[:, m0 : m0 + mw],
                        lhsT=w4[:],
                        rhs=s3[:, m0 : m0 + mw],
                        start=True,
                        stop=True,
                    )
                nc.vector.tensor_add(
                    out=out_sb[:, base + t0 : base + t0 + w],
                    in0=ps4[:, :w],
                    in1=xw[:, base + t0 : base + t0 + w],
                )
                done = b_state["tails_done"]
                done.add((t0, half))
                while (
                    b_state["flushed"] + OUT_STEP <= M_tot
                    and all(
                        (tt, hh) in done
                        for hh in (0, 1)
                        for tt in b_tails_upto(b_state["flushed"] + OUT_STEP)
                    )
                ):
                    f0 = b_state["flushed"]
                    f1 = f0 + OUT_STEP
                    for hh in (0, 1):
                        nc.sync.dma_start(
                            out=out_d[:, hh * M_tot + f0 : hh * M_tot + f1],
                            in_=out_sb[:, hh * M_tot + f0 : hh * M_tot + f1],
                        )
                    b_state["flushed"] = f1

            # ---- phase A: edge MLP + segment-sum tree (one-group
            # hysteresis), with phase-B update-MLP chunks interleaved ----
            groups = layout["groups"]

            b_bounds = list(range(0, M_tot, 512)) + [M_tot]
            b_sched = [
                (b_bounds[i], b_bounds[i + 1] - b_bounds[i], half)
                for i in range(len(b_bounds) - 1)
                for half in (0, 1)
            ]
            b_state = {"next": 0, "tails_done": set(), "flushed": 0}
            OUT_STEP = 2048

            def b_tails_upto(limit):
                return [t0 for t0, w, h in b_sched if h == 0 and t0 < limit]

            pend = []

            def maybe_emit_b(node_done):
                while pend and pend[0][4] < b_state["next"]:
                    h, t, w, s3, _ = pend.pop(0)
                    emit_b_tail(h, t, w, s3)
                while b_state["next"] < len(b_sched):
                    t0, w, half = b_sched[b_state["next"]]
                    if node_done < t0 + w:
                        break
                    pend.append(
                        (half, t0, w, emit_b_chunk(half, t0, w), b_state["next"])
                    )
                    b_state["next"] += 1
                    if node_done >= M_tot:
                        while len(pend) > 2:
                            h, t, ww, s3, _ = pend.pop(0)
                            emit_b_tail(h, t, ww, s3)
                    else:
                        break  # at most one new head per group boundary

            dma_q = [nc.sync, nc.gpsimd]
            qi = 0
            done_after = []
            pending_trees = []

            def flush_trees():
                grp_, co0_, s3v_ = pending_trees.pop(0)
                for D, m, co, no in grp_:
                    add_fn = (
                        f16_add_pool
                        if flush_trees.ridx in layout["pool_regions"]
                        else f16_add
                    )
                    _emit_tree(nc, s3v_, co - co0_, sum3, D, m, no, add_fn)
                    flush_trees.ridx += 1

            flush_trees.ridx = 0

            n_xw = 4  # xw arrives in quarters, interleaved with the rbf stream
            xw_step = -(-2 * M_tot // n_xw)
            for gi, (grp, Fg) in enumerate(groups):
                co0 = grp[0][2]
                rbf_sb = rbf_pool.tile([128, Fg], f8, tag="rbf")
                if gi >= 4:
                    step = PSUM_CHUNK
                elif gi >= 2:
                    step = PSUM_CHUNK // 2
                else:
                    step = PSUM_CHUNK // 4
                for d0 in range(0, Fg, step):
                    dw = min(step, Fg - d0)
                    q = dma_q[qi % 2]
                    q.dma_start(
                        out=rbf_sb[:, d0 : d0 + dw],
                        in_=rbf_d[:, co0 + d0 : co0 + d0 + dw],
                    )
                    qi += 1
                if gi == 0:
                    # small phase-B constants right behind the first group
                    wup = cpool.tile([128, 128], f16)
                    nc.gpsimd.dma_start(out=wup[:], in_=wup_d[:, :])
                    bias3 = cpool.tile([128, 1], f32)
                    nc.gpsimd.dma_start(out=bias3[:], in_=bias3_d[:, :])
                    w4 = cpool.tile([128, 128], f16)
                    nc.gpsimd.dma_start(out=w4[:], in_=w4_d[:, :])
                    xw = cpool.tile([128, 2 * M_tot], bf16)
                    if b2W3_nonzero:
                        degb = cpool.tile([4, 2 * M_tot], f16)
                        nc.gpsimd.dma_start(out=degb[:], in_=degb_d[:, :])
                        b2w3 = cpool.tile([2, 128], f16)
                        nc.gpsimd.dma_start(out=b2w3[:], in_=b2w3_d[:, :])
                elif 1 <= gi <= n_xw:
                    x0 = (gi - 1) * xw_step
                    x1 = min(2 * M_tot, x0 + xw_step)
                    dma_q[qi % 2].dma_start(
                        out=xw[:, x0:x1], in_=xw_d[:, x0:x1]
                    )
                    qi += 1
                s2t = s_pool.tile([128, 2 * Fg], f16, tag="s2")
                s3v = s2t[:].rearrange("p (h f) -> p h f", h=2)
                for p, lhs in ((0, lhsE), (1, lhsO)):
                    for c0, cw, on_dve in plan_by_g[(gi, p)]:
                        ps = psum_pool.tile([128, cw], f32, tag="ps")
                        for m0 in range(0, cw, MM_N):
                            mw = min(MM_N, cw - m0)
                            nc.tensor.matmul(
                                out=ps[:, m0 : m0 + mw],
                                lhsT=lhs[:],
                                rhs=rbf_sb[:, c0 + m0 : c0 + m0 + mw],
                                start=True,
                                stop=True,
                            )
                        dst = s2t[:, p * Fg + c0 : p * Fg + c0 + cw]
                        if on_dve:
                            nc.vector._custom_dve(
                                SILU_DVE_OP,
                                out=dst,
                                in0=ps[:, :cw],
                                in1=scoef[:, 2:3],
                                s0=scoef[:, 0:1],
                                s1=scoef[:, 1:2],
                                imm2=1.0,
                            )
                        else:
                            nc.scalar.activation(
                                dst, ps[:, :cw], SILU, bias=bias1[:],
                            )
                pending_trees.append((grp, co0, s3v))
                done_after.append(max(no + m for D, m, co, no in grp))
                if gi >= 1:
                    flush_trees()
                    if gi >= 2:
                        maybe_emit_b(done_after[gi - 1])

            while pending_trees:
                flush_trees()
            maybe_emit_b(M_tot)
            for h, t, w, s3, _ in pend:
                emit_b_tail(h, t, w, s3)
            f0 = b_state["flushed"]
            if f0 < M_tot:
                for hh in (0, 1):
                    nc.sync.dma_start(
                        out=out_d[:, hh * M_tot + f0 : hh * M_tot + M_tot],
                        in_=out_sb[:, hh * M_tot + f0 : hh * M_tot + M_tot],
                    )

    nc.compile()
    return nc


# --------------------------------------------------------------------------
# entry point
# --------------------------------------------------------------------------

_LAST_RESULTS = {}


def prepare(x, coord, rbf_feature, edge_index, W1, b1, W2, b2, W3, b3, W4, b4):
    """Host prep + NEFF build. Returns (nc, in_maps, meta, layout)."""
    x = np.asarray(x, np.float32)
    coord = np.asarray(coord, np.float32)
    rbf_feature = np.asarray(rbf_feature, np.float32)
    edge_index = np.asarray(edge_index)

    row = np.ascontiguousarray(edge_index[0]).astype(np.int64)
    deg = np.bincount(row, minlength=N_NODES)
    layout = _make_layout(deg)

    rbf_devs, xw_devs, lhsT_E, lhsT_O, meta, bhat = _build_host_arrays(
        x, coord, rbf_feature, edge_index, W1, b1, b4, layout
    )

    W23 = (W2.astype(np.float64) @ W3.astype(np.float64)).astype(np.float32)
    b2W3 = (b2.astype(np.float64) @ W3.astype(np.float64)).astype(np.float32)
    b2nz = bool(np.any(b2W3 != 0.0))

    wup = _blockdiag2(W23).astype(np.float16)
    w4x2 = _blockdiag2(W4.astype(np.float32)).astype(np.float16)
    identb = _blockdiag2(np.eye(EMBED, dtype=np.float32)).astype(BF16)
    b1a = (b1.astype(np.float64) + np.pi * W1[16].astype(np.float64)).astype(np.float32)
    # ACT bias = residual of the fp8 2-row bias already in the matmul
    db = (b1a.astype(np.float64) - bhat.astype(np.float64)).astype(np.float32)
    bias1 = np.concatenate([db, db]).astype(np.float32)[:, None]
    bias3 = np.concatenate([b3, b3]).astype(np.float32)[:, None]
    cs = _fit_silu_coefs(np.asarray(W1, np.float32), np.asarray(b1, np.float32))
    scoef = np.concatenate([cs, cs], axis=0).astype(np.float32)  # [128,3]

    nc = _build_nc(layout, b2nz)

    in_maps = []
    for c in range(N_CORES):
        im = {
            "rbf_dev": np.ascontiguousarray(rbf_devs[c]),
            "xw_dev": np.ascontiguousarray(xw_devs[c]),
            "lhsT_E": lhsT_E,
            "lhsT_O": lhsT_O,
            "bias1": bias1,
            "scoef": scoef,
            "Wup": wup,
            "bias3": bias3,
            "W4x2": w4x2,
            "identb": identb,
        }
        if b2nz:
            M_tot = layout["M_tot"]
            degb = np.zeros((4, 2 * M_tot), np.float32)
            nodes = np.arange(c * NPC, (c + 1) * NPC)
            st = meta["node_stream"][nodes]
            ps = meta["node_pos"][nodes]
            dg = meta["deg"][nodes].astype(np.float32)
            for q in range(N_STREAMS):
                r = q % 2
                off = 0 if q < 2 else M_tot
                degb[2 * (q // 2) + r, off + ps[st == q]] = dg[st == q]
            im["deg_dev"] = degb.astype(np.float16)
            b2w3cat = np.zeros((2, 128), np.float32)
            b2w3cat[0, 0:64] = b2W3
            b2w3cat[1, 64:128] = b2W3
            im["b2w3cat"] = b2w3cat.astype(np.float16)
        in_maps.append(im)
    return nc, in_maps, meta, layout


def postprocess(results, meta, layout):
    M_tot = layout["M_tot"]
    out = np.empty((N_NODES, EMBED), np.float32)
    ns, ps = meta["node_stream"], meta["node_pos"]
    for c in range(N_CORES):
        arr = results[c]["out_dev"]
        nodes = np.arange(c * NPC, (c + 1) * NPC)
        st = ns[nodes]
        pp = ps[nodes]
        for q in range(N_STREAMS):
            rows = slice(0, 64) if q % 2 == 0 else slice(64, 128)
            off = 0 if q < 2 else M_tot
            sel = st == q
            out[nodes[sel]] = arr[rows, off + pp[sel]].T
    return out


def kernel(x, coord, rbf_feature, edge_index, W1, b1, W2, b2, W3, b3, W4, b4):
    nc, in_maps, meta, layout = prepare(
        x, coord, rbf_feature, edge_index, W1, b1, W2, b2, W3, b3, W4, b4
    )
    res = run_bass_kernel_spmd(nc, in_maps, core_ids=list(range(N_CORES)), trace=TRACE)
    _LAST_RESULTS["res"] = res
    return postprocess(res.results, meta, layout)
